# revision 2
# baseline (speedup 1.0000x reference)
"""DenseDilatedKnnGraph kernel for 8x Trainium2 NeuronCores (v4, pairship).

Device (per core, 4096 query rows x 8192 candidates):
  - One fp32r matmul per 512-col chunk computes affine-scaled scores
    u8-domain: s_u8 = BETA*(2 a.b) + (GAMMA0 - BETA*||b||^2), built from
    65 contraction rows (64 channels + 1 bias row). 16 chunks per row-tile
    fill the 8 PSUM banks as 4 units of (A: 2 banks | B: 2 banks).
  - Act evacuates each unit's B half (1024 f32) PSUM -> SBUF.
  - DVE tensor_tensor(max)(A_psum, B_sbuf) -> uint8 (round+saturate) emits
    the pairwise column max: pool[j] = max(s[c0+j], s[c0+1024+j]).
  - DMA ships the pooled u8 matrix (4096 per row) to DRAM.
Host:
  - For each row, takes the top NPAIR_SEL pairs by u8 (argpartition), expands
    each pair to both columns, rescores candidates exactly in fp64, and takes
    ranks 0,2,...,16. Safety: at most 17 pairs can pool >= the 17th-best
    score, so top-96 pairs deterministically contain all true top-17
    columns (modulo sub-ulp rounding, covered by the wide margin).

Sharding: 32768 query rows split across 8 cores (4096 rows = half a batch).
"""
import sys
import numpy as np

sys.path.insert(0, "/opt/trn_rl_repo")

import types
try:
    from antenv import axon_hooks  # noqa: F401
except Exception:
    import antenv
    _stub = types.ModuleType("antenv.axon_hooks")
    _stub.get_axon_ntff_profile_hook = lambda: None
    sys.modules["antenv.axon_hooks"] = _stub
    antenv.axon_hooks = _stub

from concourse import bass, tile, bacc  # noqa: E402
from concourse.bass_utils import run_bass_kernel_spmd  # noqa: E402

mybir = bass.mybir
dt = mybir.dt
AF = mybir.ActivationFunctionType

B, C, N = 4, 64, 8192
KOUT = 9
NEED = 2 * KOUT - 1               # ranks 0..16
NCORES = 8
RPC = B * N // NCORES             # 4096 rows per core
RT = RPC // 128                   # 32 row-tiles per core
KR = C + 1                        # 64 channels + 1 bias row
UNITS = 4                         # units per row-tile
UCOLS = 2048                      # columns per unit (A 1024 | B 1024)
NPOOL = N // 2                    # pooled pairs per row

BETA = 170.0
GAMMA0 = 136.0
NPAIR_SEL = 96                    # pairs kept per row on host

_NC = None


def _tf32(x):
    """Round-to-nearest-even fp32 -> tf32 (10 explicit mantissa bits)."""
    x = np.ascontiguousarray(x, np.float32)
    u = x.view(np.uint32)
    keep = u & np.uint32(0xFFFFE000)
    rem = u & np.uint32(0x1FFF)
    half = np.uint32(0x1000)
    lsb = (u >> np.uint32(13)) & np.uint32(1)
    up = (rem > half) | ((rem == half) & (lsb == 1))
    return (keep + (up.astype(np.uint32) << np.uint32(13))).view(np.float32).copy()


def _build():
    nc = bacc.Bacc("TRN2", target_bir_lowering=False)
    packed_d = nc.declare_dram_parameter(
        "packed", [KR, RPC + N], dt.float32r, isOutput=False)
    pool_d = nc.declare_dram_parameter(
        "pool", [128, RT * NPOOL], dt.uint8, isOutput=True)
    with tile.TileContext(nc) as tc:
        with (
            tc.tile_pool(name="inp", bufs=1) as inp_pool,
            tc.tile_pool(name="bsb", bufs=3) as bsb_pool,
            tc.tile_pool(name="stg", bufs=4) as stg_pool,
            tc.tile_pool(name="psum", bufs=2, space="PSUM") as psum_pool,
        ):
            packed = inp_pool.tile([KR, RPC + N], dt.float32r)
            # parallel input streams, in tile-0 consumption order: A-halves
            # (unit cols [0,1024)) on the SP queue, B-halves on the Act
            # queue, lhsT row-tiles on gpsimd SWDGE.
            nc.scalar.dma_start(out=packed[:, 0:128], in_=packed_d[:, 0:128])
            for u in range(UNITS):
                c0 = RPC + u * UCOLS
                if u == 0:
                    nc.sync.dma_start(out=packed[:, c0:c0 + 512],
                                      in_=packed_d[:, c0:c0 + 512])
                    nc.sync.dma_start(out=packed[:, c0 + 512:c0 + 1024],
                                      in_=packed_d[:, c0 + 512:c0 + 1024])
                    nc.scalar.dma_start(out=packed[:, c0 + 1024:c0 + 1536],
                                        in_=packed_d[:, c0 + 1024:c0 + 1536])
                    nc.scalar.dma_start(out=packed[:, c0 + 1536:c0 + UCOLS],
                                        in_=packed_d[:, c0 + 1536:c0 + UCOLS])
                    continue
                nc.sync.dma_start(out=packed[:, c0:c0 + 1024],
                                  in_=packed_d[:, c0:c0 + 1024])
                nc.scalar.dma_start(out=packed[:, c0 + 1024:c0 + UCOLS],
                                    in_=packed_d[:, c0 + 1024:c0 + UCOLS])
                if u == 1:
                    nc.gpsimd.dma_start(out=packed[:, 128:1024],
                                        in_=packed_d[:, 128:1024])
            nc.gpsimd.dma_start(out=packed[:, 1024:RPC], in_=packed_d[:, 1024:RPC])
            lhsT_all = packed[:, 0:RPC]
            rhs_all = packed[:, RPC:]

            # PE p-state warm-up while the input DMA is in flight
            warm = inp_pool.tile([65, 512], dt.float32)
            nc.gpsimd.memset(warm[:], 0.0)
            for _ in range(2):
                pA = psum_pool.tile([128, 1024], dt.float32)
                pB = psum_pool.tile([128, 1024], dt.float32)
                nc.tensor.matmul(pA[:, 0:512], warm[:, 0:128].bitcast(dt.float32r),
                                 warm[:].bitcast(dt.float32r), start=True, stop=True)
                nc.tensor.matmul(pB[:, 0:512], warm[:, 0:128].bitcast(dt.float32r),
                                 warm[:].bitcast(dt.float32r), start=True, stop=True)

            for rt in range(RT):
                lhsT = lhsT_all[:, rt * 128:(rt + 1) * 128]
                stage = stg_pool.tile([128, UNITS * 1024], dt.uint8)
                for u in range(UNITS):
                    pA = psum_pool.tile([128, 1024], dt.float32)
                    pB = psum_pool.tile([128, 1024], dt.float32)
                    c0 = u * UCOLS
                    nc.tensor.matmul(pA[:, 0:512], lhsT[:],
                                     rhs_all[:, c0:c0 + 512],
                                     start=True, stop=True)
                    nc.tensor.matmul(pA[:, 512:1024], lhsT[:],
                                     rhs_all[:, c0 + 512:c0 + 1024],
                                     start=True, stop=True)
                    nc.tensor.matmul(pB[:, 0:512], lhsT[:],
                                     rhs_all[:, c0 + 1024:c0 + 1536],
                                     start=True, stop=True)
                    nc.tensor.matmul(pB[:, 512:1024], lhsT[:],
                                     rhs_all[:, c0 + 1536:c0 + 2048],
                                     start=True, stop=True)
                    bsb = bsb_pool.tile([128, 1024], dt.float32)
                    nc.scalar.activation(bsb[:], pB[:], AF.Copy)
                    nc.vector.tensor_tensor(stage[:, u * 1024:(u + 1) * 1024],
                                            pA[:], bsb[:], mybir.AluOpType.max)
                if rt == RT - 1:
                    # split the last tile's output so the tail drains early
                    for u in range(UNITS):
                        nc.gpsimd.dma_start(
                            out=pool_d[:, rt * NPOOL + u * 1024:
                                       rt * NPOOL + (u + 1) * 1024],
                            in_=stage[:, u * 1024:(u + 1) * 1024])
                else:
                    nc.gpsimd.dma_start(out=pool_d[:, rt * NPOOL:(rt + 1) * NPOOL],
                                        in_=stage[:])
    nc.compile()
    return nc


def _get_nc():
    global _NC
    if _NC is None:
        _NC = _build()
        try:
            zmaps = [{"packed": np.zeros((KR, RPC + N), np.float32)}
                     for _ in range(NCORES)]
            run_bass_kernel_spmd(_NC, zmaps, list(range(NCORES)))
        except Exception:
            pass
    return _NC


def _normalize(x):
    x64 = np.asarray(x).astype(np.float64)              # (B,C,N,1)
    norm = np.sqrt((x64 * x64).sum(axis=1, keepdims=True))
    pts32 = (x64 / np.maximum(norm, 1e-12)).squeeze(-1).transpose(0, 2, 1).astype(np.float32)
    return pts32


def _prep_inputs(pts32):
    in_maps = []
    gamma_b = {}
    for b in range(B):
        rb = _tf32(pts32[b]).astype(np.float64)         # (N, C) tf32 exact
        gamma_b[b] = _tf32((GAMMA0 - BETA * (rb * rb).sum(1)).astype(np.float32))
    for c in range(NCORES):
        b, h = c // 2, c % 2
        r0 = h * RPC
        packed = np.empty((KR, RPC + N), dtype=np.float32)
        packed[:C, :RPC] = _tf32(
            (2.0 * pts32[b, r0:r0 + RPC].astype(np.float64) * BETA)
            .astype(np.float32)).T
        packed[C, :RPC] = 1.0
        packed[:C, RPC:] = _tf32(pts32[b]).T
        packed[C, RPC:] = gamma_b[b]
        in_maps.append({"packed": packed})
    return in_maps


def _decode_u8(results, c):
    """Per-core pooled u8 -> [RPC, NPOOL] (row-major)."""
    v = results[c]["pool"].reshape(128, RT, NPOOL).transpose(1, 0, 2)
    return np.ascontiguousarray(v).reshape(RPC, NPOOL)


def _results_valid(results):
    """Every row's pair containing the self column must pool to 255."""
    try:
        for c in range(NCORES):
            v = results[c]["pool"]
            if v.shape != (128, RT * NPOOL):
                return False
            u8 = _decode_u8(results, c)
            r0 = (c % 2) * RPC
            rows = np.arange(RPC)
            cols = r0 + rows
            unit, off = np.divmod(cols % UCOLS, 1024)
            pair = (cols // UCOLS) * 1024 + off
            if (u8[rows, pair] == 255).mean() < 0.999:
                return False
    except Exception:
        return False
    return True


def _rescore_topk(pts32, u8_b, b):
    """Host: top pairs by u8, expand, exact rescore -> (N, KOUT) ids."""
    a = pts32[b].astype(np.float64)          # (N, C)
    sq = (a * a).sum(1)                      # (N,)
    nn = np.empty((N, KOUT), dtype=np.int32)
    # top NPAIR_SEL pairs per row
    part = np.argpartition(u8_b, NPOOL - NPAIR_SEL, axis=1)[:, NPOOL - NPAIR_SEL:]
    unit, off = np.divmod(part, 1024)
    c1 = unit * UCOLS + off                  # (N, P)
    cand = np.concatenate([c1, c1 + 1024], axis=1).astype(np.int64)  # (N, 2P)
    BLK = 2048
    for s0 in range(0, N, BLK):
        s1 = min(N, s0 + BLK)
        cb = cand[s0:s1]
        bsel = a[cb]                         # (blk, 2P, C)
        s = 2.0 * np.einsum('nc,nkc->nk', a[s0:s1], bsel) - sq[cb]
        o = np.lexsort((cb, -s), axis=1)[:, :NEED]
        top = np.take_along_axis(cb, o, axis=1)
        nn[s0:s1] = top[:, 0:NEED:2].astype(np.int32)
    return nn


def _run(x, trace=False):
    nc = _get_nc()
    pts32 = _normalize(x)
    in_maps = _prep_inputs(pts32)
    last_err = None
    for attempt in range(4):
        try:
            res = run_bass_kernel_spmd(nc, in_maps, list(range(NCORES)), trace=trace)
            if _results_valid(res.results):
                break
            last_err = RuntimeError("device returned invalid pooled values")
        except Exception as e:
            last_err = e
        import time as _time
        _time.sleep(1.0 + attempt)
    else:
        raise last_err
    nn9 = np.empty((B, N, KOUT), dtype=np.int32)
    for b in range(B):
        u8_b = np.concatenate([_decode_u8(res.results, 2 * b),
                               _decode_u8(res.results, 2 * b + 1)], axis=0)
        nn9[b] = _rescore_topk(pts32, u8_b, b)
    center = np.broadcast_to(np.arange(N, dtype=np.int32)[None, :, None],
                             (B, N, KOUT))
    edge_index = np.stack([nn9, np.ascontiguousarray(center)], axis=0)
    return edge_index, res


def kernel(x):
    edge_index, _ = _run(x, trace=False)
    return edge_index


# revision 3
# speedup vs baseline: 1.0197x; 1.0197x over previous
"""DenseDilatedKnnGraph kernel for 8x Trainium2 NeuronCores (v4, pairship).

Device (per core, 4096 query rows x 8192 candidates):
  - One fp32r matmul per 512-col chunk computes affine-scaled scores
    u8-domain: s_u8 = BETA*(2 a.b) + (GAMMA0 - BETA*||b||^2), built from
    65 contraction rows (64 channels + 1 bias row). 16 chunks per row-tile
    fill the 8 PSUM banks as 4 units of (A: 2 banks | B: 2 banks).
  - Act evacuates each unit's B half (1024 f32) PSUM -> SBUF.
  - DVE tensor_tensor(max)(A_psum, B_sbuf) -> uint8 (round+saturate) emits
    the pairwise column max: pool[j] = max(s[c0+j], s[c0+1024+j]).
  - DMA ships the pooled u8 matrix (4096 per row) to DRAM.
Host:
  - For each row, takes the top NPAIR_SEL pairs by u8 (argpartition), expands
    each pair to both columns, rescores candidates exactly in fp64, and takes
    ranks 0,2,...,16. Safety: at most 17 pairs can pool >= the 17th-best
    score, so top-96 pairs deterministically contain all true top-17
    columns (modulo sub-ulp rounding, covered by the wide margin).

Sharding: 32768 query rows split across 8 cores (4096 rows = half a batch).
"""
import sys
import numpy as np

sys.path.insert(0, "/opt/trn_rl_repo")

import types
try:
    from antenv import axon_hooks  # noqa: F401
except Exception:
    import antenv
    _stub = types.ModuleType("antenv.axon_hooks")
    _stub.get_axon_ntff_profile_hook = lambda: None
    sys.modules["antenv.axon_hooks"] = _stub
    antenv.axon_hooks = _stub

from concourse import bass, tile, bacc  # noqa: E402
from concourse.bass_utils import run_bass_kernel_spmd  # noqa: E402

mybir = bass.mybir
dt = mybir.dt
AF = mybir.ActivationFunctionType

B, C, N = 4, 64, 8192
KOUT = 9
NEED = 2 * KOUT - 1               # ranks 0..16
NCORES = 8
RPC = B * N // NCORES             # 4096 rows per core
RT = RPC // 128                   # 32 row-tiles per core
KR = C + 1                        # 64 channels + 1 bias row
UNITS = 4                         # units per row-tile
UCOLS = 2048                      # columns per unit (A 1024 | B 1024)
NPOOL = N // 2                    # pooled pairs per row

BETA = 170.0
GAMMA0 = 136.0
NPAIR_SEL = 96                    # pairs kept per row on host

_NC = None


def _bf16(x):
    """Round-to-nearest-even fp32 -> bf16 (as fp32-valued array)."""
    x = np.ascontiguousarray(x, np.float32)
    u = x.view(np.uint32)
    keep = u & np.uint32(0xFFFF0000)
    rem = u & np.uint32(0xFFFF)
    half = np.uint32(0x8000)
    lsb = (u >> np.uint32(16)) & np.uint32(1)
    up = (rem > half) | ((rem == half) & (lsb == 1))
    return (keep + (up.astype(np.uint32) << np.uint32(16))).view(np.float32).copy()


def _to_bf16_bits(x32):
    """fp32 array (already on the bf16 grid) -> uint16 bf16 bit patterns."""
    return (np.ascontiguousarray(x32, np.float32).view(np.uint32) >> 16).astype(np.uint16)


def _build():
    nc = bacc.Bacc("TRN2", target_bir_lowering=False)
    packed_d = nc.declare_dram_parameter(
        "packed", [KR, RPC + N], dt.bfloat16, isOutput=False)
    pool_d = nc.declare_dram_parameter(
        "pool", [128, RT * NPOOL], dt.uint8, isOutput=True)
    with tile.TileContext(nc) as tc:
        with (
            tc.tile_pool(name="inp", bufs=1) as inp_pool,
            tc.tile_pool(name="bsb", bufs=3) as bsb_pool,
            tc.tile_pool(name="stg", bufs=4) as stg_pool,
            tc.tile_pool(name="psum", bufs=2, space="PSUM") as psum_pool,
        ):
            packed = inp_pool.tile([KR, RPC + N], dt.bfloat16)
            # parallel input streams, in tile-0 consumption order: A-halves
            # (unit cols [0,1024)) on the SP queue, B-halves on the Act
            # queue, lhsT row-tiles on gpsimd SWDGE.
            # A-halves on the SP HWDGE queue, B-halves + small lhsT head on
            # the Act HWDGE queue, bulk lhsT on SP after the rhs stream.
            nc.gpsimd.dma_start(out=packed[:, 0:128], in_=packed_d[:, 0:128])
            for u in range(UNITS):
                c0 = RPC + u * UCOLS
                if u == 0:
                    for q in range(2):
                        nc.sync.dma_start(
                            out=packed[:, c0 + q * 512:c0 + (q + 1) * 512],
                            in_=packed_d[:, c0 + q * 512:c0 + (q + 1) * 512])
                        nc.scalar.dma_start(
                            out=packed[:, c0 + 1024 + q * 512:c0 + 1024 + (q + 1) * 512],
                            in_=packed_d[:, c0 + 1024 + q * 512:c0 + 1024 + (q + 1) * 512])
                    continue
                nc.sync.dma_start(out=packed[:, c0:c0 + 1024],
                                  in_=packed_d[:, c0:c0 + 1024])
                if u == 1:
                    nc.scalar.dma_start(out=packed[:, c0 + 1024:c0 + UCOLS],
                                        in_=packed_d[:, c0 + 1024:c0 + UCOLS])
            # B-halves of units 2-3 as one strided transfer (fewer issues
            # on the Act queue)
            bview_sb = packed[:, RPC + 2 * UCOLS:RPC + N].rearrange(
                "p (u c) -> p u c", c=UCOLS)[:, :, 1024:UCOLS]
            bview_d = packed_d[:, RPC + 2 * UCOLS:RPC + N].rearrange(
                "p (u c) -> p u c", c=UCOLS)[:, :, 1024:UCOLS]
            nc.scalar.dma_start(out=bview_sb, in_=bview_d)
            nc.sync.dma_start(out=packed[:, 128:1024], in_=packed_d[:, 128:1024])
            nc.sync.dma_start(out=packed[:, 1024:RPC], in_=packed_d[:, 1024:RPC])
            lhsT_all = packed[:, 0:RPC]
            rhs_all = packed[:, RPC:]

            # PE p-state warm-up while the input DMA is in flight
            warm = inp_pool.tile([65, 512], dt.float32)
            nc.gpsimd.memset(warm[:], 0.0)
            for _ in range(2):
                pA = psum_pool.tile([128, 1024], dt.float32)
                pB = psum_pool.tile([128, 1024], dt.float32)
                nc.tensor.matmul(pA[:, 0:512], warm[:, 0:64].bitcast(dt.bfloat16),
                                 warm[:, 64:320].bitcast(dt.bfloat16),
                                 start=True, stop=True)
                nc.tensor.matmul(pB[:, 0:512], warm[:, 0:64].bitcast(dt.bfloat16),
                                 warm[:, 64:320].bitcast(dt.bfloat16),
                                 start=True, stop=True)

            for rt in range(RT):
                lhsT = lhsT_all[:, rt * 128:(rt + 1) * 128]
                stage = stg_pool.tile([128, UNITS * 1024], dt.uint8)
                for u in range(UNITS):
                    pA = psum_pool.tile([128, 1024], dt.float32)
                    pB = psum_pool.tile([128, 1024], dt.float32)
                    c0 = u * UCOLS
                    nc.tensor.matmul(pA[:, 0:512], lhsT[:],
                                     rhs_all[:, c0:c0 + 512],
                                     start=True, stop=True)
                    nc.tensor.matmul(pA[:, 512:1024], lhsT[:],
                                     rhs_all[:, c0 + 512:c0 + 1024],
                                     start=True, stop=True)
                    nc.tensor.matmul(pB[:, 0:512], lhsT[:],
                                     rhs_all[:, c0 + 1024:c0 + 1536],
                                     start=True, stop=True)
                    nc.tensor.matmul(pB[:, 512:1024], lhsT[:],
                                     rhs_all[:, c0 + 1536:c0 + 2048],
                                     start=True, stop=True)
                    bsb = bsb_pool.tile([128, 1024], dt.float32)
                    if rt == 0:
                        # 512-wide ops on tile 0: same pair semantics, but
                        # each starts as soon as its 512-col DMA lands
                        for q in range(2):
                            sl = slice(q * 512, (q + 1) * 512)
                            nc.scalar.activation(bsb[:, sl], pB[:, sl], AF.Copy)
                            nc.vector.tensor_tensor(
                                stage[:, u * 1024 + q * 512:u * 1024 + (q + 1) * 512],
                                pA[:, sl], bsb[:, sl], mybir.AluOpType.max)
                    else:
                        nc.scalar.activation(bsb[:], pB[:], AF.Copy)
                        nc.vector.tensor_tensor(stage[:, u * 1024:(u + 1) * 1024],
                                                pA[:], bsb[:], mybir.AluOpType.max)
                if rt == RT - 1:
                    # split the last tile's output on the (idle by now) SP
                    # HWDGE queue so the tail drains early
                    for u in range(UNITS):
                        nc.sync.dma_start(
                            out=pool_d[:, rt * NPOOL + u * 1024:
                                       rt * NPOOL + (u + 1) * 1024],
                            in_=stage[:, u * 1024:(u + 1) * 1024])
                else:
                    nc.gpsimd.dma_start(out=pool_d[:, rt * NPOOL:(rt + 1) * NPOOL],
                                        in_=stage[:])
    nc.compile()
    return nc


def _get_nc():
    global _NC
    if _NC is None:
        _NC = _build()
        try:
            zmaps = [{"packed": np.zeros((KR, RPC + N), np.uint16)}
                     for _ in range(NCORES)]
            run_bass_kernel_spmd(_NC, zmaps, list(range(NCORES)))
        except Exception:
            pass
    return _NC


def _normalize(x):
    x64 = np.asarray(x).astype(np.float64)              # (B,C,N,1)
    norm = np.sqrt((x64 * x64).sum(axis=1, keepdims=True))
    pts32 = (x64 / np.maximum(norm, 1e-12)).squeeze(-1).transpose(0, 2, 1).astype(np.float32)
    return pts32


def _prep_inputs(pts32):
    in_maps = []
    gamma_b = {}
    for b in range(B):
        rb = _bf16(pts32[b]).astype(np.float64)         # (N, C) bf16 exact
        gamma_b[b] = _bf16((GAMMA0 - BETA * (rb * rb).sum(1)).astype(np.float32))
    for c in range(NCORES):
        b, h = c // 2, c % 2
        r0 = h * RPC
        packed = np.empty((KR, RPC + N), dtype=np.float32)
        packed[:C, :RPC] = _bf16(
            (2.0 * pts32[b, r0:r0 + RPC].astype(np.float64) * BETA)
            .astype(np.float32)).T
        packed[C, :RPC] = 1.0
        packed[:C, RPC:] = _bf16(pts32[b]).T
        packed[C, RPC:] = gamma_b[b]
        in_maps.append({"packed": _to_bf16_bits(packed)})
    return in_maps


def _decode_u8(results, c):
    """Per-core pooled u8 -> [RPC, NPOOL] (row-major)."""
    v = results[c]["pool"].reshape(128, RT, NPOOL).transpose(1, 0, 2)
    return np.ascontiguousarray(v).reshape(RPC, NPOOL)


def _results_valid(results):
    """Every row's pair containing the self column must pool to 255."""
    try:
        for c in range(NCORES):
            v = results[c]["pool"]
            if v.shape != (128, RT * NPOOL):
                return False
            u8 = _decode_u8(results, c)
            r0 = (c % 2) * RPC
            rows = np.arange(RPC)
            cols = r0 + rows
            unit, off = np.divmod(cols % UCOLS, 1024)
            pair = (cols // UCOLS) * 1024 + off
            if (u8[rows, pair] == 255).mean() < 0.999:
                return False
    except Exception:
        return False
    return True


def _rescore_topk(pts32, u8_b, b):
    """Host: top pairs by u8, expand, exact rescore -> (N, KOUT) ids."""
    a = pts32[b].astype(np.float64)          # (N, C)
    sq = (a * a).sum(1)                      # (N,)
    nn = np.empty((N, KOUT), dtype=np.int32)
    # top NPAIR_SEL pairs per row
    part = np.argpartition(u8_b, NPOOL - NPAIR_SEL, axis=1)[:, NPOOL - NPAIR_SEL:]
    unit, off = np.divmod(part, 1024)
    c1 = unit * UCOLS + off                  # (N, P)
    cand = np.concatenate([c1, c1 + 1024], axis=1).astype(np.int64)  # (N, 2P)
    BLK = 2048
    for s0 in range(0, N, BLK):
        s1 = min(N, s0 + BLK)
        cb = cand[s0:s1]
        bsel = a[cb]                         # (blk, 2P, C)
        s = 2.0 * np.einsum('nc,nkc->nk', a[s0:s1], bsel) - sq[cb]
        o = np.lexsort((cb, -s), axis=1)[:, :NEED]
        top = np.take_along_axis(cb, o, axis=1)
        nn[s0:s1] = top[:, 0:NEED:2].astype(np.int32)
    return nn


def _run(x, trace=False):
    nc = _get_nc()
    pts32 = _normalize(x)
    in_maps = _prep_inputs(pts32)
    last_err = None
    for attempt in range(4):
        try:
            res = run_bass_kernel_spmd(nc, in_maps, list(range(NCORES)), trace=trace)
            if _results_valid(res.results):
                break
            last_err = RuntimeError("device returned invalid pooled values")
        except Exception as e:
            last_err = e
        import time as _time
        _time.sleep(1.0 + attempt)
    else:
        raise last_err
    nn9 = np.empty((B, N, KOUT), dtype=np.int32)
    for b in range(B):
        u8_b = np.concatenate([_decode_u8(res.results, 2 * b),
                               _decode_u8(res.results, 2 * b + 1)], axis=0)
        nn9[b] = _rescore_topk(pts32, u8_b, b)
    center = np.broadcast_to(np.arange(N, dtype=np.int32)[None, :, None],
                             (B, N, KOUT))
    edge_index = np.stack([nn9, np.ascontiguousarray(center)], axis=0)
    return edge_index, res


def kernel(x):
    edge_index, _ = _run(x, trace=False)
    return edge_index


# revision 4
# speedup vs baseline: 1.0294x; 1.0095x over previous
"""DenseDilatedKnnGraph kernel for 8x Trainium2 NeuronCores (v4, pairship).

Device (per core, 4096 query rows x 8192 candidates):
  - One fp32r matmul per 512-col chunk computes affine-scaled scores
    u8-domain: s_u8 = BETA*(2 a.b) + (GAMMA0 - BETA*||b||^2), built from
    65 contraction rows (64 channels + 1 bias row). 16 chunks per row-tile
    fill the 8 PSUM banks as 4 units of (A: 2 banks | B: 2 banks).
  - Act evacuates each unit's B half (1024 f32) PSUM -> SBUF.
  - DVE tensor_tensor(max)(A_psum, B_sbuf) -> uint8 (round+saturate) emits
    the pairwise column max: pool[j] = max(s[c0+j], s[c0+1024+j]).
  - DMA ships the pooled u8 matrix (4096 per row) to DRAM.
Host:
  - For each row, takes the top NPAIR_SEL pairs by u8 (argpartition), expands
    each pair to both columns, rescores candidates exactly in fp64, and takes
    ranks 0,2,...,16. Safety: at most 17 pairs can pool >= the 17th-best
    score, so top-96 pairs deterministically contain all true top-17
    columns (modulo sub-ulp rounding, covered by the wide margin).

Sharding: 32768 query rows split across 8 cores (4096 rows = half a batch).
"""
import sys
import numpy as np

sys.path.insert(0, "/opt/trn_rl_repo")

import types
try:
    from antenv import axon_hooks  # noqa: F401
except Exception:
    import antenv
    _stub = types.ModuleType("antenv.axon_hooks")
    _stub.get_axon_ntff_profile_hook = lambda: None
    sys.modules["antenv.axon_hooks"] = _stub
    antenv.axon_hooks = _stub

from concourse import bass, tile, bacc  # noqa: E402
from concourse.bass_utils import run_bass_kernel_spmd  # noqa: E402

mybir = bass.mybir
dt = mybir.dt
AF = mybir.ActivationFunctionType

B, C, N = 4, 64, 8192
KOUT = 9
NEED = 2 * KOUT - 1               # ranks 0..16
NCORES = 8
RPC = B * N // NCORES             # 4096 rows per core
RT = RPC // 128                   # 32 row-tiles per core
KR = C + 1                        # 64 channels + 1 bias row
UNITS = 4                         # units per row-tile
UCOLS = 2048                      # columns per unit (A 1024 | B 1024)
NPOOL = N // 2                    # pooled pairs per row

BETA = 170.0
GAMMA0 = 136.0
NPAIR_SEL = 96                    # pairs kept per row on host

_NC = None


def _bf16(x):
    """Round-to-nearest-even fp32 -> bf16 (as fp32-valued array)."""
    x = np.ascontiguousarray(x, np.float32)
    u = x.view(np.uint32)
    keep = u & np.uint32(0xFFFF0000)
    rem = u & np.uint32(0xFFFF)
    half = np.uint32(0x8000)
    lsb = (u >> np.uint32(16)) & np.uint32(1)
    up = (rem > half) | ((rem == half) & (lsb == 1))
    return (keep + (up.astype(np.uint32) << np.uint32(16))).view(np.float32).copy()


def _to_bf16_bits(x32):
    """fp32 array (already on the bf16 grid) -> uint16 bf16 bit patterns."""
    return (np.ascontiguousarray(x32, np.float32).view(np.uint32) >> 16).astype(np.uint16)


def _build():
    nc = bacc.Bacc("TRN2", target_bir_lowering=False)
    packed_d = nc.declare_dram_parameter(
        "packed", [KR, RPC + N], dt.bfloat16, isOutput=False)
    pool_d = nc.declare_dram_parameter(
        "pool", [128, RT * NPOOL], dt.uint8, isOutput=True)
    with tile.TileContext(nc) as tc:
        with (
            tc.tile_pool(name="inp", bufs=1) as inp_pool,
            tc.tile_pool(name="bsb", bufs=3) as bsb_pool,
            tc.tile_pool(name="stg", bufs=4) as stg_pool,
            tc.tile_pool(name="psum", bufs=2, space="PSUM") as psum_pool,
        ):
            packed = inp_pool.tile([KR, RPC + N], dt.bfloat16)
            # packed layout: [lhsT tile0 (128) | rhs (8192) | lhsT rest]
            # -> the first DMA covers everything tile-0 unit-0 needs.
            # HWDGE issues (~625ns, globally serialized) dominate startup,
            # so the input is 6 large transfers split across SP/Act queues.
            nc.sync.dma_start(out=packed[:, 0:128 + UCOLS],
                              in_=packed_d[:, 0:128 + UCOLS])
            nc.sync.dma_start(out=packed[:, 128 + UCOLS:128 + 2 * UCOLS],
                              in_=packed_d[:, 128 + UCOLS:128 + 2 * UCOLS])
            nc.scalar.dma_start(out=packed[:, 128 + 2 * UCOLS:128 + 3 * UCOLS],
                                in_=packed_d[:, 128 + 2 * UCOLS:128 + 3 * UCOLS])
            nc.scalar.dma_start(out=packed[:, 128 + 3 * UCOLS:128 + N],
                                in_=packed_d[:, 128 + 3 * UCOLS:128 + N])
            nc.sync.dma_start(out=packed[:, 128 + N:128 + N + 15 * 128],
                              in_=packed_d[:, 128 + N:128 + N + 15 * 128])
            nc.sync.dma_start(out=packed[:, 128 + N + 15 * 128:],
                              in_=packed_d[:, 128 + N + 15 * 128:])
            rhs_all = packed[:, 128:128 + N]

            # PE p-state warm-up while the input DMA is in flight
            warm = inp_pool.tile([65, 512], dt.float32)
            nc.gpsimd.memset(warm[:], 0.0)
            for _ in range(1):
                pA = psum_pool.tile([128, 1024], dt.float32)
                pB = psum_pool.tile([128, 1024], dt.float32)
                nc.tensor.matmul(pA[:, 0:256], warm[:, 0:64].bitcast(dt.bfloat16),
                                 warm[:, 64:192].bitcast(dt.bfloat16),
                                 start=True, stop=True)
                nc.tensor.matmul(pB[:, 0:256], warm[:, 0:64].bitcast(dt.bfloat16),
                                 warm[:, 64:192].bitcast(dt.bfloat16),
                                 start=True, stop=True)

            for rt in range(RT):
                lhsT = packed[:, 0:128] if rt == 0 else \
                    packed[:, N + rt * 128:N + (rt + 1) * 128]
                stage = stg_pool.tile([128, UNITS * 1024], dt.uint8)
                for u in range(UNITS):
                    pA = psum_pool.tile([128, 1024], dt.float32)
                    pB = psum_pool.tile([128, 1024], dt.float32)
                    c0 = u * UCOLS
                    nc.tensor.matmul(pA[:, 0:512], lhsT[:],
                                     rhs_all[:, c0:c0 + 512],
                                     start=True, stop=True)
                    nc.tensor.matmul(pA[:, 512:1024], lhsT[:],
                                     rhs_all[:, c0 + 512:c0 + 1024],
                                     start=True, stop=True)
                    nc.tensor.matmul(pB[:, 0:512], lhsT[:],
                                     rhs_all[:, c0 + 1024:c0 + 1536],
                                     start=True, stop=True)
                    nc.tensor.matmul(pB[:, 512:1024], lhsT[:],
                                     rhs_all[:, c0 + 1536:c0 + 2048],
                                     start=True, stop=True)
                    bsb = bsb_pool.tile([128, 1024], dt.float32)
                    nc.scalar.activation(bsb[:], pB[:], AF.Copy)
                    nc.vector.tensor_tensor(stage[:, u * 1024:(u + 1) * 1024],
                                            pA[:], bsb[:], mybir.AluOpType.max)
                if rt == RT - 1:
                    # split the last tile's output on the (idle by now) SP
                    # HWDGE queue so the tail drains early
                    for u in range(UNITS):
                        nc.sync.dma_start(
                            out=pool_d[:, rt * NPOOL + u * 1024:
                                       rt * NPOOL + (u + 1) * 1024],
                            in_=stage[:, u * 1024:(u + 1) * 1024])
                else:
                    nc.gpsimd.dma_start(out=pool_d[:, rt * NPOOL:(rt + 1) * NPOOL],
                                        in_=stage[:])
    nc.compile()
    return nc


def _get_nc():
    global _NC
    if _NC is None:
        _NC = _build()
        try:
            zmaps = [{"packed": np.zeros((KR, RPC + N), np.uint16)}
                     for _ in range(NCORES)]
            run_bass_kernel_spmd(_NC, zmaps, list(range(NCORES)))
        except Exception:
            pass
    return _NC


def _normalize(x):
    x64 = np.asarray(x).astype(np.float64)              # (B,C,N,1)
    norm = np.sqrt((x64 * x64).sum(axis=1, keepdims=True))
    pts32 = (x64 / np.maximum(norm, 1e-12)).squeeze(-1).transpose(0, 2, 1).astype(np.float32)
    return pts32


def _prep_inputs(pts32):
    in_maps = []
    gamma_b = {}
    for b in range(B):
        rb = _bf16(pts32[b]).astype(np.float64)         # (N, C) bf16 exact
        gamma_b[b] = _bf16((GAMMA0 - BETA * (rb * rb).sum(1)).astype(np.float32))
    for c in range(NCORES):
        b, h = c // 2, c % 2
        r0 = h * RPC
        packed = np.empty((KR, RPC + N), dtype=np.float32)
        lhsT = np.empty((KR, RPC), dtype=np.float32)
        lhsT[:C] = _bf16(
            (2.0 * pts32[b, r0:r0 + RPC].astype(np.float64) * BETA)
            .astype(np.float32)).T
        lhsT[C] = 1.0
        packed[:, 0:128] = lhsT[:, 0:128]
        packed[:C, 128:128 + N] = _bf16(pts32[b]).T
        packed[C, 128:128 + N] = gamma_b[b]
        packed[:, 128 + N:] = lhsT[:, 128:]
        in_maps.append({"packed": _to_bf16_bits(packed)})
    return in_maps


def _decode_u8(results, c):
    """Per-core pooled u8 -> [RPC, NPOOL] (row-major)."""
    v = results[c]["pool"].reshape(128, RT, NPOOL).transpose(1, 0, 2)
    return np.ascontiguousarray(v).reshape(RPC, NPOOL)


def _results_valid(results):
    """Every row's pair containing the self column must pool to 255."""
    try:
        for c in range(NCORES):
            v = results[c]["pool"]
            if v.shape != (128, RT * NPOOL):
                return False
            u8 = _decode_u8(results, c)
            r0 = (c % 2) * RPC
            rows = np.arange(RPC)
            cols = r0 + rows
            unit, off = np.divmod(cols % UCOLS, 1024)
            pair = (cols // UCOLS) * 1024 + off
            if (u8[rows, pair] == 255).mean() < 0.999:
                return False
    except Exception:
        return False
    return True


def _rescore_topk(pts32, u8_b, b):
    """Host: top pairs by u8, expand, exact rescore -> (N, KOUT) ids."""
    a = pts32[b].astype(np.float64)          # (N, C)
    sq = (a * a).sum(1)                      # (N,)
    nn = np.empty((N, KOUT), dtype=np.int32)
    # top NPAIR_SEL pairs per row
    part = np.argpartition(u8_b, NPOOL - NPAIR_SEL, axis=1)[:, NPOOL - NPAIR_SEL:]
    unit, off = np.divmod(part, 1024)
    c1 = unit * UCOLS + off                  # (N, P)
    cand = np.concatenate([c1, c1 + 1024], axis=1).astype(np.int64)  # (N, 2P)
    BLK = 2048
    for s0 in range(0, N, BLK):
        s1 = min(N, s0 + BLK)
        cb = cand[s0:s1]
        bsel = a[cb]                         # (blk, 2P, C)
        s = 2.0 * np.einsum('nc,nkc->nk', a[s0:s1], bsel) - sq[cb]
        o = np.lexsort((cb, -s), axis=1)[:, :NEED]
        top = np.take_along_axis(cb, o, axis=1)
        nn[s0:s1] = top[:, 0:NEED:2].astype(np.int32)
    return nn


def _run(x, trace=False):
    nc = _get_nc()
    pts32 = _normalize(x)
    in_maps = _prep_inputs(pts32)
    last_err = None
    for attempt in range(4):
        try:
            res = run_bass_kernel_spmd(nc, in_maps, list(range(NCORES)), trace=trace)
            if _results_valid(res.results):
                break
            last_err = RuntimeError("device returned invalid pooled values")
        except Exception as e:
            last_err = e
        import time as _time
        _time.sleep(1.0 + attempt)
    else:
        raise last_err
    nn9 = np.empty((B, N, KOUT), dtype=np.int32)
    for b in range(B):
        u8_b = np.concatenate([_decode_u8(res.results, 2 * b),
                               _decode_u8(res.results, 2 * b + 1)], axis=0)
        nn9[b] = _rescore_topk(pts32, u8_b, b)
    center = np.broadcast_to(np.arange(N, dtype=np.int32)[None, :, None],
                             (B, N, KOUT))
    edge_index = np.stack([nn9, np.ascontiguousarray(center)], axis=0)
    return edge_index, res


def kernel(x):
    edge_index, _ = _run(x, trace=False)
    return edge_index
